# revision 21
# baseline (speedup 1.0000x reference)
"""AnalyticGaussianVelocity Trainium2 kernel, 8 NeuronCores.

Math (reference):
    a=t, b=1-t
    logit_n = -(1/(2b^2)) * (|x|^2 - 2a x.y_n + a^2 |y_n|^2)
    v = -(1/b) x + (1 + a/b) * softmax(logit) @ dataset

Device-side per core (dataset sharded along N, padded 50000->51200, 6400/core,
free-dim chunks 12x512+256):
    G_n   = x.y_n + u*(|y_n|^2 - 512)   u=-a/2    (4 f32r matmuls -> PSUM +
                                                   a 5th 3-row split-bf16
                                                   rank-1 matmul)
    logit'_n = c1 * G_n, c1 = a/b^2
    m     = max_n logit'_n              (ACT Copy drain -> DVE chunk maxes)
    P_n   = exp(logit'_n - m)           (ACT exp, fp8-e4m3 out, f32 accum -> l)
    S^T   = sum_n P_n y_n               (fp8 DoubleRow matmuls: dataset is the
                                         stationary side split into e4m3
                                         hi+lo, P^T pairs are the moving side
                                         via u16 XBAR transposes of the fp8 P;
                                         out is S^T [d, b], 2x PE throughput
                                         vs bf16)
Host combine (flash-attention style) over the 8 core shards:
    M = max_c m_c; w = exp(m_c - M); S = sum w*S_c; L = sum w*l_c
    v = -(1/b) x + (1 + a/b) * S / L

Padding rows are the constant 2048.0 -> giant |y|^2 -> logit ~ -1e7 -> w 0.
The DoubleRow pairing groups n = 256*s + 2p + i (p = partition, i = k-group);
the u16 transpose of the fp8 P pairs adjacent n automatically, and the
dataset hi/lo tensors are pre-interleaved on the host to match.
"""

import numpy as np
import ml_dtypes

import concourse.bass as bass
from concourse import bacc
import concourse.mybir as mybir
import concourse.tile as tile
from concourse.bass_utils import run_bass_kernel_spmd

F32 = mybir.dt.float32
F32R = mybir.dt.float32r
F16 = mybir.dt.float16
BF16 = mybir.dt.bfloat16
FP8 = mybir.dt.float8e4
U16 = mybir.dt.uint16
BF = ml_dtypes.bfloat16
E4 = ml_dtypes.float8_e4m3
DR = mybir.MatmulPerfMode.DoubleRow

B, D, N = 1024, 512, 50000
NCORES = 8
NPAD = 51200                      # 8 * 6400, multiple of 2048
NSH = NPAD // NCORES              # 6400 per core
KD = D // 128                     # 4 contraction tiles for logits matmul
BT = B // 128                     # 8 batch tiles
CHUNKS = [256] + [512] * 12       # free-dim chunks of NSH (>=256: f32r rate;
                                  # small first chunk starts mm1 sooner)
# chunk pairs sharing one P tile / one XBAR transpose
PAIRS = [(0, 1), (2, 3), (4, 5), (6, 7), (8, 9), (10, 11), (12,)]
NSLAB = NSH // 256                # 25 DoubleRow slabs (256 n each)
DSL = D // 128                    # 4 d-slices for the S^T matmuls
PADVAL = 2048.0
X = mybir.AxisListType.X

GBUFS = 16                        # G chunks in flight (13/tile + slack)


def _build(combine=False):  # combine kept for test.py compat; host always combines
    nc = bacc.Bacc("TRN2", target_bir_lowering=False, debug=False,
                   num_devices=NCORES, dynamic_dma_scratch_size=512)

    xT = nc.declare_dram_parameter("xT", [KD, 128, B], F16, isOutput=False)
    dsT = nc.declare_dram_parameter("dsT", [KD, 128, NSH], F16, isOutput=False)
    ds_hi = nc.declare_dram_parameter("ds_hi", [128, NSLAB, 2, D], FP8,
                                      isOutput=False)
    ds_lo = nc.declare_dram_parameter("ds_lo", [128, NSLAB, 2, D], FP8,
                                      isOutput=False)
    c1d = nc.declare_dram_parameter("c1", [128, BT], F32, isOutput=False)
    r1l = nc.declare_dram_parameter("r1_lhsT", [3, B], BF16, isOutput=False)
    r1r = nc.declare_dram_parameter("r1_rhs", [3, NSH], BF16, isOutput=False)

    S_out = nc.declare_dram_parameter("S_outT", [BT, 128, DSL, 128], F32,
                                      isOutput=True)
    m_out = nc.declare_dram_parameter("m_out", [128, BT], F32, isOutput=True)
    l_out = nc.declare_dram_parameter("l_out", [128, BT], F32, isOutput=True)

    nch = len(CHUNKS)
    coff = np.concatenate([[0], np.cumsum(CHUNKS)])
    pair_of = {}
    off_in_pair = {}
    pair_last = {}
    pair_slab0 = {}
    for p, members in enumerate(PAIRS):
        o = 0
        for c in members:
            pair_of[c] = p
            off_in_pair[c] = o
            o += CHUNKS[c]
        pair_last[p] = members[-1]
        pair_slab0[p] = int(coff[members[0]]) // 256
        assert o <= 1024

    with tile.TileContext(nc) as tc:
        with (
            tc.tile_pool(name="res", bufs=1) as res,
            tc.tile_pool(name="gpool", bufs=GBUFS) as gpool,
            tc.tile_pool(name="ppool", bufs=3) as ppool,
            tc.tile_pool(name="small", bufs=2) as small,
            tc.tile_pool(name="ptpool", bufs=6) as ptpool,
            tc.tile_pool(name="gps", bufs=4, space="PSUM") as gps,
            tc.tile_pool(name="spsum", bufs=2, space="PSUM") as spsum,
        ):
            # ---- residents (DMA in first-use order; smalls off sync queue) --
            c1_t = res.tile([128, BT], F32, tag="c1")
            nc.gpsimd.dma_start(c1_t[:], c1d[:])
            r1l_t = res.tile([3, B], BF16, tag="r1l")
            nc.gpsimd.dma_start(r1l_t[:], r1l[:])
            r1r_t = res.tile([3, NSH], BF16, tag="r1r")
            nc.gpsimd.dma_start(r1r_t[:], r1r[:])

            xT_r = res.tile([128, KD, B], F16, tag="xT_r")
            xT_re = xT.ap().rearrange("k p b -> p k b")
            # tile 0 leads; tile 1 (PRE) follows right after chunk 0
            nc.sync.dma_start(xT_r[:, :, 0:128], xT_re[:, :, 0:128])

            dsT_r = res.tile([128, KD, NSH], F16, tag="dsT_r")
            dsT_re = dsT.ap().rearrange("k p n -> p k n")
            dhi_t = res.tile([128, NSLAB, 2, D], FP8, tag="dhi")
            dlo_t = res.tile([128, NSLAB, 2, D], FP8, tag="dlo")

            def load_ds_chunk(c):
                o = int(coff[c])
                w = CHUNKS[c]
                # one DMA per chunk covering all 4 k-tiles: HWDGE descriptor
                # generation is a serialized global resource (~625ns/DMA)
                nc.sync.dma_start(dsT_r[:, :, o:o + w], dsT_re[:, :, o:o + w])

            def load_ds8(s0, s1):
                nc.sync.dma_start(dhi_t[:, s0:s1], ds_hi.ap()[:, s0:s1])
                nc.sync.dma_start(dlo_t[:, s0:s1], ds_lo.ap()[:, s0:s1])

            # the DMA transfer device is globally serial: order transfers by
            # first use. dsT chunks feed phase 0 progressively; ds8 halves
            # must land before mm2(0) finishes (phase-1 end); xT tiles 2..7
            # are the least urgent (first used at phase 2).
            load_ds_chunk(0)
            nc.sync.dma_start(xT_r[:, :, 128:256], xT_re[:, :, 128:256])
            for c in range(1, nch):
                load_ds_chunk(c)
            nc.sync.dma_start(xT_r[:, :, 256:384], xT_re[:, :, 256:384])
            load_ds8(0, 13)
            # ds8 slabs 13+ and xT tiles 3..7 are emitted inside phase 1,
            # interleaved with the first pair transposes, so the serial DMA
            # device services transposes at their deadlines (see loop).

            m_sb = res.tile([128, BT], F32, tag="m_sb")
            l_sb = res.tile([128, BT], F32, tag="l_sb")

            state = {}

            def emit_mm1_chunk(i, c, w):
                o = int(coff[c])
                gch, gmax = state[i][:2]
                g_ps = gps.tile([128, 512], F32, tag="gps")
                for k in range(KD):
                    nc.tensor.matmul(
                        g_ps[:, :w],
                        xT_r[:, k, i * 128:(i + 1) * 128],
                        dsT_r[:, k, o:o + w],
                        start=(k == 0), stop=False,
                    )
                nc.tensor.matmul(
                    g_ps[:, :w],
                    r1l_t[:, i * 128:(i + 1) * 128],
                    r1r_t[:, o:o + w],
                    start=False, stop=True,
                )
                G_c = gpool.tile([128, 512], F32, tag="G")
                nc.scalar.activation(G_c[:, :w], g_ps[:, :w],
                                     mybir.ActivationFunctionType.Copy)
                gch[c] = G_c
                maxq.add((i, c))

            maxq = set()

            def emit_max(i, c):
                if (i, c) not in maxq:
                    return
                maxq.discard((i, c))
                gch, gmax = state[i][:2]
                w = CHUNKS[c]
                nc.vector.reduce_max(gmax[:, c:c + 1], gch[c][:, :w], axis=X,
                                     op=mybir.AluOpType.max)

            def emit_exp_head(i):
                gch, gmax = state[i]
                gm = small.tile([128, 1], F32, tag="gm")
                nc.vector.reduce_max(gm[:], gmax[:], axis=X,
                                     op=mybir.AluOpType.max)
                nc.vector.tensor_mul(m_sb[:, i:i + 1], gm[:], c1_t[:, i:i + 1])
                nb = small.tile([128, 1], F32, tag="nb")
                nc.vector.tensor_scalar_mul(nb[:], m_sb[:, i:i + 1], -1.0)
                if i == BT - 1:
                    nc.gpsimd.dma_start(m_out.ap(), m_sb[:])
                lparts = small.tile([128, nch], F32, tag="lp")
                S_ps = spsum.tile([128, DSL, 128], F32, tag="S")
                state[i] = (gch, gmax, nb, lparts, S_ps)

            ptt = {}
            pcur = {}

            def emit_exp_chunk(i, c):
                gch, gmax, nb, lparts, S_ps = state[i]
                w = CHUNKS[c]
                pair = pair_of[c]
                off = off_in_pair[c]
                G_c = gch.pop(c)
                # chunks of a pair share one P tile so their u16-pair XBAR
                # transpose is a single DMA (HWDGE gen is globally serialized)
                if off == 0:
                    P2 = ppool.tile([128, 1024], FP8, tag="P")
                    pcur[(i, pair)] = P2
                else:
                    P2 = pcur[(i, pair)]
                nc.scalar.activation(
                    P2[:, off:off + w], G_c[:, :w],
                    mybir.ActivationFunctionType.Exp,
                    bias=nb[:], scale=c1_t[:, i:i + 1],
                    accum_out=lparts[:, c:c + 1],
                )
                if c == pair_last[pair]:
                    wp = off + w
                    pt = ptpool.tile([128, 4, 128], U16, tag="pt")
                    nc.sync.dma_start(pt[:, :wp // 256, :],
                                      P2.bitcast(U16)[:, :wp // 2],
                                      transpose=True)
                    ptt[(i, pair)] = pt
                    pcur.pop((i, pair))

            def emit_mm2_chunk(i, c, w):
                gch, gmax, nb, lparts, S_ps = state[i]
                pair = pair_of[c]
                pt = ptt[(i, pair)]
                s0 = int(coff[c]) // 256
                # the whole [128, 4, 128] tile is one PSUM bank = one zero
                # region: a single accumulation group (start on the global
                # first matmul, stop on the global last) — per-slice groups
                # would re-zero each other's partial sums.
                for j in range(w // 256):
                    s = s0 + j
                    blk = s - pair_slab0[pair]
                    ptj = pt[:, blk, :]
                    mov = ptj.bitcast(FP8).rearrange("p (b i) -> p i b", i=2)
                    for h, dst in enumerate((dhi_t, dlo_t)):
                        for e in range(DSL):
                            nc.tensor.matmul(
                                S_ps[:, e, :],
                                dst[:, s, :, e * 128:(e + 1) * 128],
                                mov,
                                start=(s == 0 and h == 0 and e == 0),
                                stop=(s == NSLAB - 1 and h == 1 and e == DSL - 1),
                                perf_mode=DR,
                                skip_group_check=True,
                            )
                if c == pair_last[pair]:
                    ptt.pop((i, pair))

            pending_stores = []

            def emit_mm2_tail(i):
                _, _, _, lparts, S_ps = state.pop(i)
                nc.vector.reduce_sum(l_sb[:, i:i + 1], lparts[:], axis=X,
                                     op=mybir.AluOpType.add)
                if i == BT - 1:
                    nc.gpsimd.dma_start(l_out.ap(), l_sb[:])
                S_sb = gpool.tile([128, 512], F32, tag="G")
                S_flat = S_ps[:].rearrange("p e b -> p (e b)")
                if i == BT - 1:
                    # split the final drain into pipelined halves
                    nc.vector.tensor_copy(S_sb[:, :256], S_flat[:, :256])
                    nc.sync.dma_start(
                        S_out.ap().rearrange("i p e b -> p i (e b)")[:, i, :256],
                        S_sb[:, :256])
                    nc.vector.tensor_copy(S_sb[:, 256:], S_flat[:, 256:])
                    nc.sync.dma_start(
                        S_out.ap().rearrange("i p e b -> p i (e b)")[:, i, 256:],
                        S_sb[:, 256:])
                else:
                    nc.vector.tensor_copy(S_sb[:], S_flat)
                    pending_stores.append((i, S_sb))

            def flush_stores():
                while pending_stores:
                    i, S_sb = pending_stores.pop(0)
                    nc.sync.dma_start(
                        S_out.ap().rearrange("i p e b -> p i (e b)")[:, i, :],
                        S_sb[:])

            def alloc_tile_state(i):
                gmax = small.tile([128, nch], F32, tag="gmax")
                state[i] = ({}, gmax)

            # software-pipelined, per phase i: mm1 of tile i leads, exp of
            # tile i-1 tracks it, mm2 of tile i-1 lags by LAG chunks so the
            # phase-boundary chain (exp_head -> exp -> DMA-transpose) is
            # covered by mm1 work on the PE
            LAG = 6
            PRE = 3
            alloc_tile_state(0)
            if BT > 1:
                alloc_tile_state(1)
            for c, w in enumerate(CHUNKS):
                emit_mm1_chunk(0, c, w)
                if c < PRE:
                    emit_mm1_chunk(1, c, w)
                if c > 0:
                    emit_max(0, c - 1)
            emit_max(0, nch - 1)
            for i in range(1, BT):
                if i not in state:
                    alloc_tile_state(i)
                emit_exp_head(i - 1)
                sh = PRE if i == 1 else 0   # step-1 mm1 chunks shifted by PRE
                for k in range(sh):
                    emit_max(i, k)
                for c in range(nch):
                    if c + sh < nch:
                        emit_mm1_chunk(i, c + sh, CHUNKS[c + sh])
                    emit_exp_chunk(i - 1, c)
                    if i == 1 and c == 1:
                        load_ds8(13, 19)
                    if i == 1 and c == 3:
                        load_ds8(19, NSLAB)
                    if i == 1 and c == 5:
                        nc.sync.dma_start(xT_r[:, :, 384:B],
                                          xT_re[:, :, 384:B])
                    if c == 2:
                        flush_stores()
                    if c >= LAG:
                        emit_mm2_chunk(i - 1, c - LAG, CHUNKS[c - LAG])
                    if c > 0 and c - 1 + sh < nch:
                        emit_max(i, c - 1 + sh)
                emit_max(i, nch - 1)
                if i == BT - 1:
                    emit_exp_head(BT - 1)
                    emit_exp_chunk(BT - 1, 0)
                    emit_exp_chunk(BT - 1, 1)
                for c in range(nch - LAG, nch):
                    emit_mm2_chunk(i - 1, c, CHUNKS[c])
                emit_mm2_tail(i - 1)
            i = BT - 1
            for c, w in enumerate(CHUNKS):
                if c + 2 < nch:
                    emit_exp_chunk(i, c + 2)
                if c == 2:
                    flush_stores()
                emit_mm2_chunk(i, c, w)
            emit_mm2_tail(i)
            flush_stores()

    nc.compile()
    return nc


_NC_CACHE = {}


def _get_nc():
    if "nc" not in _NC_CACHE:
        _NC_CACHE["nc"] = _build()
    return _NC_CACHE["nc"]


def _split_bf16(v):
    hi = v.astype(np.float32).astype(BF)
    lo = (v.astype(np.float64) - hi.astype(np.float64)).astype(np.float32).astype(BF)
    return hi, lo


def _prep_inputs(x_t, t, dataset):
    x_t = np.asarray(x_t, dtype=np.float32)
    t = np.asarray(t, dtype=np.float32)
    dataset = np.asarray(dataset, dtype=np.float32)

    a = t.astype(np.float64)
    b = 1.0 - a
    c1 = np.ascontiguousarray(
        (a / (b * b)).astype(np.float32).reshape(BT, 128).T)

    dsp = np.full((NPAD, D), PADVAL, dtype=np.float32)
    dsp[:N] = dataset
    # beta must be consistent with the f16-rounded y used by mm1
    dsp16 = dsp.astype(np.float16).astype(np.float64)
    dsnc = ((dsp16 ** 2).sum(1) - float(D)).astype(np.float32)

    uu = -a / 2.0
    u_hi, u_lo = _split_bf16(uu)
    r1_lhsT = np.stack([u_hi, u_lo, u_hi]).astype(BF)       # (3, B)
    v_hi, v_lo = _split_bf16(dsnc.astype(np.float64))
    r1_rhs_full = np.stack([v_hi, v_hi, v_lo]).astype(BF)   # (3, NPAD)

    xT = np.ascontiguousarray(x_t.T).astype(np.float16).reshape(KD, 128, B)
    dsT_full = np.ascontiguousarray(dsp.T).astype(np.float16)  # (D, NPAD)

    # fp8 copies feed mm2 only; pad rows carry zero softmax weight but 2048
    # overflows e4m3 (max 448) into NaN, so zero them there.
    dsp8 = dsp.copy()
    dsp8[N:] = 0.0
    hi8 = dsp8.astype(E4)                                   # (NPAD, D)
    lo8 = (dsp8.astype(np.float64) - hi8.astype(np.float64)).astype(
        np.float32).astype(E4)

    in_maps = []
    for c in range(NCORES):
        sl = slice(c * NSH, (c + 1) * NSH)
        # DoubleRow pairing: n_local = 256*s + 2*p + i -> [p, s, i, d]
        hi_sh = np.ascontiguousarray(
            hi8[sl].reshape(NSLAB, 128, 2, D).transpose(1, 0, 2, 3))
        lo_sh = np.ascontiguousarray(
            lo8[sl].reshape(NSLAB, 128, 2, D).transpose(1, 0, 2, 3))
        im = {
            "xT": xT,
            "dsT": np.ascontiguousarray(dsT_full[:, sl]).reshape(KD, 128, NSH),
            "ds_hi": hi_sh,
            "ds_lo": lo_sh,
            "c1": c1,
            "r1_lhsT": r1_lhsT,
            "r1_rhs": np.ascontiguousarray(r1_rhs_full[:, sl]),
        }
        in_maps.append(im)
    return in_maps


def _combine_host(results, x_t, t):
    a = t.astype(np.float64)
    b = 1.0 - a
    m_c = np.stack([np.asarray(r["m_out"], dtype=np.float64).T.reshape(-1)
                    for r in results])                      # (8, B)
    l_c = np.stack([np.asarray(r["l_out"], dtype=np.float64).T.reshape(-1)
                    for r in results])                      # (8, B)
    # S_outT[i, p, e, bb] = S[b = i*128+bb, d = e*128+p]
    S_c = np.stack([np.asarray(r["S_outT"], dtype=np.float64)
                    .transpose(0, 3, 2, 1).reshape(B, D)
                    for r in results])                      # (8, B, D)
    M = m_c.max(0)
    w = np.exp(m_c - M)                                     # (8, B)
    S = np.einsum("cb,cbd->bd", w, S_c)
    L = (w * l_c).sum(0)
    wd = S / L[:, None]
    v = (-1.0 / b)[:, None] * x_t.astype(np.float64) \
        + (1.0 + a / b)[:, None] * wd
    return v.astype(np.float32)


def run_full(x_t, t, dataset, trace=False):
    nc = _get_nc()
    in_maps = _prep_inputs(x_t, t, dataset)
    res = run_bass_kernel_spmd(nc, in_maps, core_ids=list(range(NCORES)),
                               trace=trace)
    v = _combine_host(res.results, np.asarray(x_t, np.float32),
                      np.asarray(t, np.float32))
    return v, res


def kernel(x_t, t, dataset):
    v, _ = run_full(x_t, t, dataset)
    return v


# revision 24
# speedup vs baseline: 1.0586x; 1.0586x over previous
"""AnalyticGaussianVelocity Trainium2 kernel, 8 NeuronCores.

Math (reference):
    a=t, b=1-t
    logit_n = -(1/(2b^2)) * (|x|^2 - 2a x.y_n + a^2 |y_n|^2)
    v = -(1/b) x + (1 + a/b) * softmax(logit) @ dataset

Device-side per core (dataset sharded along N, padded 50000->51200, 6400/core,
free-dim chunks 12x512+256):
    G_n   = x.y_n + u*(|y_n|^2 - 512)   u=-a/2    (4 f32r matmuls -> PSUM +
                                                   a 5th 3-row split-bf16
                                                   rank-1 matmul)
    logit'_n = c1 * G_n, c1 = a/b^2
    m     = max_n logit'_n              (ACT Copy drain -> DVE chunk maxes)
    P_n   = exp(logit'_n - m)           (ACT exp, fp8-e4m3 out, f32 accum -> l)
    S^T   = sum_n P_n y_n               (fp8 DoubleRow matmuls: dataset is the
                                         stationary side split into e4m3
                                         hi+lo, P^T pairs are the moving side
                                         via u16 XBAR transposes of the fp8 P;
                                         out is S^T [d, b], 2x PE throughput
                                         vs bf16)
Host combine (flash-attention style) over the 8 core shards:
    M = max_c m_c; w = exp(m_c - M); S = sum w*S_c; L = sum w*l_c
    v = -(1/b) x + (1 + a/b) * S / L

Padding rows are the constant 2048.0 -> giant |y|^2 -> logit ~ -1e7 -> w 0.
The DoubleRow pairing groups n = 256*s + 2p + i (p = partition, i = k-group);
the u16 transpose of the fp8 P pairs adjacent n automatically, and the
dataset hi/lo tensors are pre-interleaved on the host to match.
"""

import numpy as np
import ml_dtypes

import concourse.bass as bass
from concourse import bacc
import concourse.mybir as mybir
import concourse.tile as tile
from concourse.bass_utils import run_bass_kernel_spmd

F32 = mybir.dt.float32
F32R = mybir.dt.float32r
F16 = mybir.dt.float16
BF16 = mybir.dt.bfloat16
FP8 = mybir.dt.float8e4
U16 = mybir.dt.uint16
BF = ml_dtypes.bfloat16
E4 = ml_dtypes.float8_e4m3
DR = mybir.MatmulPerfMode.DoubleRow

B, D, N = 1024, 512, 50000
NCORES = 8
NPAD = 51200                      # 8 * 6400, multiple of 2048
NSH = NPAD // NCORES              # 6400 per core
KD = D // 128                     # 4 contraction tiles for logits matmul
BT = B // 128                     # 8 batch tiles
CHUNKS = [256] + [512] * 12       # free-dim chunks of NSH (>=256: f32r rate;
                                  # small first chunk starts mm1 sooner)
# chunk pairs sharing one P tile / one XBAR transpose
PAIRS = [(0, 1), (2, 3), (4, 5), (6, 7), (8, 9), (10, 11), (12,)]
NSLAB = NSH // 256                # 25 DoubleRow slabs (256 n each)
DSL = D // 128                    # 4 d-slices for the S^T matmuls
PADVAL = 2048.0
X = mybir.AxisListType.X

GBUFS = 19                        # G chunks in flight (13/tile + slack)


def _build(combine=False):  # combine kept for test.py compat; host always combines
    nc = bacc.Bacc("TRN2", target_bir_lowering=False, debug=False,
                   num_devices=NCORES, dynamic_dma_scratch_size=512)

    xT = nc.declare_dram_parameter("xT", [KD, 128, B], F16, isOutput=False)
    dsT = nc.declare_dram_parameter("dsT", [KD, 128, NSH], F16, isOutput=False)
    ds_hi = nc.declare_dram_parameter("ds_hi", [128, NSLAB, 2, D], FP8,
                                      isOutput=False)
    ds_lo = nc.declare_dram_parameter("ds_lo", [128, NSLAB, 2, D], FP8,
                                      isOutput=False)
    c1d = nc.declare_dram_parameter("c1", [128, BT], F32, isOutput=False)
    r1l = nc.declare_dram_parameter("r1_lhsT", [3, B], BF16, isOutput=False)
    r1r = nc.declare_dram_parameter("r1_rhs", [3, NSH], BF16, isOutput=False)

    S_out = nc.declare_dram_parameter("S_outT", [BT, 128, DSL, 128], F32,
                                      isOutput=True)
    m_out = nc.declare_dram_parameter("m_out", [128, BT], F32, isOutput=True)
    l_out = nc.declare_dram_parameter("l_out", [128, BT], F32, isOutput=True)

    nch = len(CHUNKS)
    coff = np.concatenate([[0], np.cumsum(CHUNKS)])
    pair_of = {}
    off_in_pair = {}
    pair_last = {}
    pair_slab0 = {}
    for p, members in enumerate(PAIRS):
        o = 0
        for c in members:
            pair_of[c] = p
            off_in_pair[c] = o
            o += CHUNKS[c]
        pair_last[p] = members[-1]
        pair_slab0[p] = int(coff[members[0]]) // 256
        assert o <= 1024

    with tile.TileContext(nc) as tc:
        with (
            tc.tile_pool(name="res", bufs=1) as res,
            tc.tile_pool(name="gpool", bufs=GBUFS) as gpool,
            tc.tile_pool(name="ppool", bufs=8) as ppool,
            tc.tile_pool(name="small", bufs=2) as small,
            tc.tile_pool(name="ptpool", bufs=8) as ptpool,
            tc.tile_pool(name="gps", bufs=4, space="PSUM") as gps,
            tc.tile_pool(name="spsum", bufs=2, space="PSUM") as spsum,
        ):
            # ---- residents (DMA in first-use order; smalls off sync queue) --
            c1_t = res.tile([128, BT], F32, tag="c1")
            nc.gpsimd.dma_start(c1_t[:], c1d[:])
            r1l_t = res.tile([3, B], BF16, tag="r1l")
            nc.gpsimd.dma_start(r1l_t[:], r1l[:])
            r1r_t = res.tile([3, NSH], BF16, tag="r1r")
            nc.gpsimd.dma_start(r1r_t[:], r1r[:])

            xT_r = res.tile([128, KD, B], F16, tag="xT_r")
            xT_re = xT.ap().rearrange("k p b -> p k b")
            # tile 0 leads; tile 1 (PRE) follows right after chunk 0
            nc.sync.dma_start(xT_r[:, :, 0:128], xT_re[:, :, 0:128])

            dsT_r = res.tile([128, KD, NSH], F16, tag="dsT_r")
            dsT_re = dsT.ap().rearrange("k p n -> p k n")
            dhi_t = res.tile([128, NSLAB, 2, D], FP8, tag="dhi")
            dlo_t = res.tile([128, NSLAB, 2, D], FP8, tag="dlo")

            def load_ds_chunk(c):
                o = int(coff[c])
                w = CHUNKS[c]
                # one DMA per chunk covering all 4 k-tiles: HWDGE descriptor
                # generation is a serialized global resource (~625ns/DMA)
                nc.sync.dma_start(dsT_r[:, :, o:o + w], dsT_re[:, :, o:o + w])

            def load_ds8(s0, s1):
                nc.sync.dma_start(dhi_t[:, s0:s1], ds_hi.ap()[:, s0:s1])
                nc.sync.dma_start(dlo_t[:, s0:s1], ds_lo.ap()[:, s0:s1])

            # the DMA transfer device is globally serial: order transfers by
            # first use. dsT chunks feed phase 0 progressively; ds8 halves
            # must land before mm2(0) finishes (phase-1 end); xT tiles 2..7
            # are the least urgent (first used at phase 2).
            load_ds_chunk(0)
            nc.sync.dma_start(xT_r[:, :, 128:256], xT_re[:, :, 128:256])
            for c in range(1, nch):
                load_ds_chunk(c)
            nc.sync.dma_start(xT_r[:, :, 256:384], xT_re[:, :, 256:384])
            load_ds8(0, 13)
            load_ds8(13, NSLAB)
            nc.sync.dma_start(xT_r[:, :, 384:B], xT_re[:, :, 384:B])

            m_sb = res.tile([128, BT], F32, tag="m_sb")
            l_sb = res.tile([128, BT], F32, tag="l_sb")

            state = {}

            def emit_mm1_chunk(i, c, w):
                o = int(coff[c])
                gch, gmax = state[i][:2]
                g_ps = gps.tile([128, 512], F32, tag="gps")
                for k in range(KD):
                    nc.tensor.matmul(
                        g_ps[:, :w],
                        xT_r[:, k, i * 128:(i + 1) * 128],
                        dsT_r[:, k, o:o + w],
                        start=(k == 0), stop=False,
                    )
                nc.tensor.matmul(
                    g_ps[:, :w],
                    r1l_t[:, i * 128:(i + 1) * 128],
                    r1r_t[:, o:o + w],
                    start=False, stop=True,
                )
                G_c = gpool.tile([128, 512], F32, tag="G")
                nc.scalar.activation(G_c[:, :w], g_ps[:, :w],
                                     mybir.ActivationFunctionType.Copy)
                gch[c] = G_c
                maxq.add((i, c))

            maxq = set()

            def emit_max(i, c):
                if (i, c) not in maxq:
                    return
                maxq.discard((i, c))
                gch, gmax = state[i][:2]
                w = CHUNKS[c]
                nc.vector.reduce_max(gmax[:, c:c + 1], gch[c][:, :w], axis=X,
                                     op=mybir.AluOpType.max)

            def emit_exp_head(i):
                gch, gmax = state[i]
                gm = small.tile([128, 1], F32, tag="gm")
                nc.vector.reduce_max(gm[:], gmax[:], axis=X,
                                     op=mybir.AluOpType.max)
                nc.vector.tensor_mul(m_sb[:, i:i + 1], gm[:], c1_t[:, i:i + 1])
                nb = small.tile([128, 1], F32, tag="nb")
                nc.vector.tensor_scalar_mul(nb[:], m_sb[:, i:i + 1], -1.0)
                if i == BT - 1:
                    nc.gpsimd.dma_start(m_out.ap(), m_sb[:])
                lparts = small.tile([128, nch], F32, tag="lp")
                S_ps = spsum.tile([128, DSL, 128], F32, tag="S")
                state[i] = (gch, gmax, nb, lparts, S_ps)

            ptt = {}
            pcur = {}

            def emit_exp_chunk(i, c):
                gch, gmax, nb, lparts, S_ps = state[i]
                w = CHUNKS[c]
                pair = pair_of[c]
                off = off_in_pair[c]
                G_c = gch.pop(c)
                # chunks of a pair share one P tile so their u16-pair XBAR
                # transpose is a single DMA (HWDGE gen is globally serialized)
                if off == 0:
                    P2 = ppool.tile([128, 1024], FP8, tag="P")
                    pcur[(i, pair)] = P2
                else:
                    P2 = pcur[(i, pair)]
                nc.scalar.activation(
                    P2[:, off:off + w], G_c[:, :w],
                    mybir.ActivationFunctionType.Exp,
                    bias=nb[:], scale=c1_t[:, i:i + 1],
                    accum_out=lparts[:, c:c + 1],
                )
                if c == pair_last[pair]:
                    wp = off + w
                    pt = ptpool.tile([128, 4, 128], U16, tag="pt")
                    nc.sync.dma_start(pt[:, :wp // 256, :],
                                      P2.bitcast(U16)[:, :wp // 2],
                                      transpose=True)
                    ptt[(i, pair)] = pt
                    pcur.pop((i, pair))

            def emit_mm2_chunk(i, c, w):
                gch, gmax, nb, lparts, S_ps = state[i]
                pair = pair_of[c]
                pt = ptt[(i, pair)]
                s0 = int(coff[c]) // 256
                # the whole [128, 4, 128] tile is one PSUM bank = one zero
                # region: a single accumulation group (start on the global
                # first matmul, stop on the global last) — per-slice groups
                # would re-zero each other's partial sums.
                for j in range(w // 256):
                    s = s0 + j
                    blk = s - pair_slab0[pair]
                    ptj = pt[:, blk, :]
                    mov = ptj.bitcast(FP8).rearrange("p (b i) -> p i b", i=2)
                    for h, dst in enumerate((dhi_t, dlo_t)):
                        for e in range(DSL):
                            nc.tensor.matmul(
                                S_ps[:, e, :],
                                dst[:, s, :, e * 128:(e + 1) * 128],
                                mov,
                                start=(s == 0 and h == 0 and e == 0),
                                stop=(s == NSLAB - 1 and h == 1 and e == DSL - 1),
                                perf_mode=DR,
                                skip_group_check=True,
                            )
                if c == pair_last[pair]:
                    ptt.pop((i, pair))

            pending_stores = []

            def emit_mm2_tail(i):
                _, _, _, lparts, S_ps = state.pop(i)
                nc.vector.reduce_sum(l_sb[:, i:i + 1], lparts[:], axis=X,
                                     op=mybir.AluOpType.add)
                if i == BT - 1:
                    nc.gpsimd.dma_start(l_out.ap(), l_sb[:])
                S_sb = gpool.tile([128, 512], F32, tag="G")
                S_flat = S_ps[:].rearrange("p e b -> p (e b)")
                if i == BT - 1:
                    # split the final drain into pipelined halves
                    nc.vector.tensor_copy(S_sb[:, :256], S_flat[:, :256])
                    nc.sync.dma_start(
                        S_out.ap().rearrange("i p e b -> p i (e b)")[:, i, :256],
                        S_sb[:, :256])
                    nc.vector.tensor_copy(S_sb[:, 256:], S_flat[:, 256:])
                    nc.sync.dma_start(
                        S_out.ap().rearrange("i p e b -> p i (e b)")[:, i, 256:],
                        S_sb[:, 256:])
                else:
                    nc.vector.tensor_copy(S_sb[:], S_flat)
                    pending_stores.append((i, S_sb))

            def flush_stores():
                while pending_stores:
                    i, S_sb = pending_stores.pop(0)
                    nc.sync.dma_start(
                        S_out.ap().rearrange("i p e b -> p i (e b)")[:, i, :],
                        S_sb[:])

            def alloc_tile_state(i):
                gmax = small.tile([128, nch], F32, tag="gmax")
                state[i] = ({}, gmax)

            # software-pipelined, per phase i: mm1 of tile i leads, exp of
            # tile i-1 tracks it, mm2 of tile i-1 lags by LAG chunks so the
            # phase-boundary chain (exp_head -> exp -> DMA-transpose) is
            # covered by mm1 work on the PE
            LAG = 6
            import os
            LAG1 = int(os.environ.get("LAG1", "6"))   # phase-1 lag (tile 0)
            PRE = 3
            alloc_tile_state(0)
            if BT > 1:
                alloc_tile_state(1)
            for c, w in enumerate(CHUNKS):
                emit_mm1_chunk(0, c, w)
                if c < PRE:
                    emit_mm1_chunk(1, c, w)
                if c > 0:
                    emit_max(0, c - 1)
            emit_max(0, nch - 1)
            for i in range(1, BT):
                if i not in state:
                    alloc_tile_state(i)
                emit_exp_head(i - 1)
                sh = PRE if i == 1 else 0   # step-1 mm1 chunks shifted by PRE
                for k in range(sh):
                    emit_max(i, k)
                lag = LAG1 if i == 1 else LAG
                for c in range(nch):
                    if c + sh < nch:
                        emit_mm1_chunk(i, c + sh, CHUNKS[c + sh])
                    emit_exp_chunk(i - 1, c)
                    if c == 2:
                        flush_stores()
                    if c >= lag:
                        emit_mm2_chunk(i - 1, c - lag, CHUNKS[c - lag])
                    if c > 0 and c - 1 + sh < nch:
                        emit_max(i, c - 1 + sh)
                emit_max(i, nch - 1)
                if i == BT - 1:
                    emit_exp_head(BT - 1)
                    emit_exp_chunk(BT - 1, 0)
                    emit_exp_chunk(BT - 1, 1)
                for c in range(nch - lag, nch):
                    emit_mm2_chunk(i - 1, c, CHUNKS[c])
                emit_mm2_tail(i - 1)
            i = BT - 1
            for c, w in enumerate(CHUNKS):
                if c + 2 < nch:
                    emit_exp_chunk(i, c + 2)
                if c == 2:
                    flush_stores()
                emit_mm2_chunk(i, c, w)
            emit_mm2_tail(i)
            flush_stores()

    nc.compile()
    return nc


_NC_CACHE = {}


def _get_nc():
    if "nc" not in _NC_CACHE:
        _NC_CACHE["nc"] = _build()
    return _NC_CACHE["nc"]


def _split_bf16(v):
    hi = v.astype(np.float32).astype(BF)
    lo = (v.astype(np.float64) - hi.astype(np.float64)).astype(np.float32).astype(BF)
    return hi, lo


def _prep_inputs(x_t, t, dataset):
    x_t = np.asarray(x_t, dtype=np.float32)
    t = np.asarray(t, dtype=np.float32)
    dataset = np.asarray(dataset, dtype=np.float32)

    a = t.astype(np.float64)
    b = 1.0 - a
    c1 = np.ascontiguousarray(
        (a / (b * b)).astype(np.float32).reshape(BT, 128).T)

    dsp = np.full((NPAD, D), PADVAL, dtype=np.float32)
    dsp[:N] = dataset
    # beta must be consistent with the f16-rounded y used by mm1
    dsp16 = dsp.astype(np.float16).astype(np.float64)
    dsnc = ((dsp16 ** 2).sum(1) - float(D)).astype(np.float32)

    uu = -a / 2.0
    u_hi, u_lo = _split_bf16(uu)
    r1_lhsT = np.stack([u_hi, u_lo, u_hi]).astype(BF)       # (3, B)
    v_hi, v_lo = _split_bf16(dsnc.astype(np.float64))
    r1_rhs_full = np.stack([v_hi, v_hi, v_lo]).astype(BF)   # (3, NPAD)

    xT = np.ascontiguousarray(x_t.T).astype(np.float16).reshape(KD, 128, B)
    dsT_full = np.ascontiguousarray(dsp.T).astype(np.float16)  # (D, NPAD)

    # fp8 copies feed mm2 only; pad rows carry zero softmax weight but 2048
    # overflows e4m3 (max 448) into NaN, so zero them there.
    dsp8 = dsp.copy()
    dsp8[N:] = 0.0
    hi8 = dsp8.astype(E4)                                   # (NPAD, D)
    lo8 = (dsp8.astype(np.float64) - hi8.astype(np.float64)).astype(
        np.float32).astype(E4)

    in_maps = []
    for c in range(NCORES):
        sl = slice(c * NSH, (c + 1) * NSH)
        # DoubleRow pairing: n_local = 256*s + 2*p + i -> [p, s, i, d]
        hi_sh = np.ascontiguousarray(
            hi8[sl].reshape(NSLAB, 128, 2, D).transpose(1, 0, 2, 3))
        lo_sh = np.ascontiguousarray(
            lo8[sl].reshape(NSLAB, 128, 2, D).transpose(1, 0, 2, 3))
        im = {
            "xT": xT,
            "dsT": np.ascontiguousarray(dsT_full[:, sl]).reshape(KD, 128, NSH),
            "ds_hi": hi_sh,
            "ds_lo": lo_sh,
            "c1": c1,
            "r1_lhsT": r1_lhsT,
            "r1_rhs": np.ascontiguousarray(r1_rhs_full[:, sl]),
        }
        in_maps.append(im)
    return in_maps


def _combine_host(results, x_t, t):
    a = t.astype(np.float64)
    b = 1.0 - a
    m_c = np.stack([np.asarray(r["m_out"], dtype=np.float64).T.reshape(-1)
                    for r in results])                      # (8, B)
    l_c = np.stack([np.asarray(r["l_out"], dtype=np.float64).T.reshape(-1)
                    for r in results])                      # (8, B)
    # S_outT[i, p, e, bb] = S[b = i*128+bb, d = e*128+p]
    S_c = np.stack([np.asarray(r["S_outT"], dtype=np.float64)
                    .transpose(0, 3, 2, 1).reshape(B, D)
                    for r in results])                      # (8, B, D)
    M = m_c.max(0)
    w = np.exp(m_c - M)                                     # (8, B)
    S = np.einsum("cb,cbd->bd", w, S_c)
    L = (w * l_c).sum(0)
    wd = S / L[:, None]
    v = (-1.0 / b)[:, None] * x_t.astype(np.float64) \
        + (1.0 + a / b)[:, None] * wd
    return v.astype(np.float32)


def run_full(x_t, t, dataset, trace=False):
    nc = _get_nc()
    in_maps = _prep_inputs(x_t, t, dataset)
    res = run_bass_kernel_spmd(nc, in_maps, core_ids=list(range(NCORES)),
                               trace=trace)
    v = _combine_host(res.results, np.asarray(x_t, np.float32),
                      np.asarray(t, np.float32))
    return v, res


def kernel(x_t, t, dataset):
    v, _ = run_full(x_t, t, dataset)
    return v


# revision 25
# speedup vs baseline: 1.1103x; 1.0487x over previous
"""AnalyticGaussianVelocity Trainium2 kernel, 8 NeuronCores.

Math (reference):
    a=t, b=1-t
    logit_n = -(1/(2b^2)) * (|x|^2 - 2a x.y_n + a^2 |y_n|^2)
    v = -(1/b) x + (1 + a/b) * softmax(logit) @ dataset

Device-side per core (dataset sharded along N, padded 50000->51200, 6400/core,
free-dim chunks 12x512+256):
    G_n   = x.y_n + u*(|y_n|^2 - 512)   u=-a/2    (4 f32r matmuls -> PSUM +
                                                   a 5th 3-row split-bf16
                                                   rank-1 matmul)
    logit'_n = c1 * G_n, c1 = a/b^2
    m     = max_n logit'_n              (ACT Copy drain -> DVE chunk maxes)
    P_n   = exp(logit'_n - m)           (ACT exp, fp8-e4m3 out, f32 accum -> l)
    S^T   = sum_n P_n y_n               (fp8 DoubleRow matmuls: dataset is the
                                         stationary side split into e4m3
                                         hi+lo, P^T pairs are the moving side
                                         via u16 XBAR transposes of the fp8 P;
                                         out is S^T [d, b], 2x PE throughput
                                         vs bf16)
Host combine (flash-attention style) over the 8 core shards:
    M = max_c m_c; w = exp(m_c - M); S = sum w*S_c; L = sum w*l_c
    v = -(1/b) x + (1 + a/b) * S / L

Padding rows are the constant 2048.0 -> giant |y|^2 -> logit ~ -1e7 -> w 0.
The DoubleRow pairing groups n = 256*s + 2p + i (p = partition, i = k-group);
the u16 transpose of the fp8 P pairs adjacent n automatically, and the
dataset hi/lo tensors are pre-interleaved on the host to match.
"""

import numpy as np
import ml_dtypes

import concourse.bass as bass
from concourse import bacc
import concourse.mybir as mybir
import concourse.tile as tile
from concourse.bass_utils import run_bass_kernel_spmd

F32 = mybir.dt.float32
F32R = mybir.dt.float32r
F16 = mybir.dt.float16
BF16 = mybir.dt.bfloat16
FP8 = mybir.dt.float8e4
U16 = mybir.dt.uint16
BF = ml_dtypes.bfloat16
E4 = ml_dtypes.float8_e4m3
DR = mybir.MatmulPerfMode.DoubleRow

B, D, N = 1024, 512, 50000
NCORES = 8
NPAD = 51200                      # 8 * 6400, multiple of 2048
NSH = NPAD // NCORES              # 6400 per core
KD = D // 128                     # 4 contraction tiles for logits matmul
BT = B // 128                     # 8 batch tiles
CHUNKS = [256] + [512] * 12       # free-dim chunks of NSH (>=256: f32r rate;
                                  # small first chunk starts mm1 sooner)
# chunk pairs sharing one P tile / one XBAR transpose
PAIRS = [(0, 1), (2, 3), (4, 5), (6, 7), (8, 9), (10, 11), (12,)]
NSLAB = NSH // 256                # 25 DoubleRow slabs (256 n each)
DSL = D // 128                    # 4 d-slices for the S^T matmuls
PADVAL = 2048.0
X = mybir.AxisListType.X

GBUFS = 19                        # G chunks in flight (13/tile + slack)


def _build(combine=False):  # combine kept for test.py compat; host always combines
    nc = bacc.Bacc("TRN2", target_bir_lowering=False, debug=False,
                   num_devices=NCORES, dynamic_dma_scratch_size=512)

    xT = nc.declare_dram_parameter("xT", [KD, 128, B], F16, isOutput=False)
    dsT = nc.declare_dram_parameter("dsT", [KD, 128, NSH], F16, isOutput=False)
    ds_hi = nc.declare_dram_parameter("ds_hi", [128, NSLAB, 2, D], FP8,
                                      isOutput=False)
    ds_lo = nc.declare_dram_parameter("ds_lo", [128, NSLAB, 2, D], FP8,
                                      isOutput=False)
    c1d = nc.declare_dram_parameter("c1", [128, BT], F32, isOutput=False)
    r1l = nc.declare_dram_parameter("r1_lhsT", [3, B], BF16, isOutput=False)
    r1r = nc.declare_dram_parameter("r1_rhs", [3, NSH], BF16, isOutput=False)

    S_out = nc.declare_dram_parameter("S_outT", [BT, 128, DSL, 128], F32,
                                      isOutput=True)
    m_out = nc.declare_dram_parameter("m_out", [128, BT], F32, isOutput=True)
    l_out = nc.declare_dram_parameter("l_out", [128, BT], F32, isOutput=True)

    nch = len(CHUNKS)
    coff = np.concatenate([[0], np.cumsum(CHUNKS)])
    pair_of = {}
    off_in_pair = {}
    pair_last = {}
    pair_slab0 = {}
    for p, members in enumerate(PAIRS):
        o = 0
        for c in members:
            pair_of[c] = p
            off_in_pair[c] = o
            o += CHUNKS[c]
        pair_last[p] = members[-1]
        pair_slab0[p] = int(coff[members[0]]) // 256
        assert o <= 1024

    with tile.TileContext(nc) as tc:
        with (
            tc.tile_pool(name="res", bufs=1) as res,
            tc.tile_pool(name="gpool", bufs=GBUFS) as gpool,
            tc.tile_pool(name="ppool", bufs=8) as ppool,
            tc.tile_pool(name="small", bufs=2) as small,
            tc.tile_pool(name="ptpool", bufs=8) as ptpool,
            tc.tile_pool(name="gps", bufs=4, space="PSUM") as gps,
            tc.tile_pool(name="spsum", bufs=2, space="PSUM") as spsum,
        ):
            # ---- residents (DMA in first-use order; smalls off sync queue) --
            c1_t = res.tile([128, BT], F32, tag="c1")
            nc.gpsimd.dma_start(c1_t[:], c1d[:])
            r1l_t = res.tile([3, B], BF16, tag="r1l")
            nc.gpsimd.dma_start(r1l_t[:], r1l[:])
            r1r_t = res.tile([3, NSH], BF16, tag="r1r")
            nc.gpsimd.dma_start(r1r_t[:], r1r[:])

            xT_r = res.tile([128, KD, B], F16, tag="xT_r")
            xT_re = xT.ap().rearrange("k p b -> p k b")
            # tile 0 leads; tile 1 (PRE) follows right after chunk 0
            nc.sync.dma_start(xT_r[:, :, 0:128], xT_re[:, :, 0:128])

            dsT_r = res.tile([128, KD, NSH], F16, tag="dsT_r")
            dsT_re = dsT.ap().rearrange("k p n -> p k n")
            dhi_t = res.tile([128, NSLAB, 2, D], FP8, tag="dhi")
            dlo_t = res.tile([128, NSLAB, 2, D], FP8, tag="dlo")

            def load_ds_chunk(c):
                o = int(coff[c])
                w = CHUNKS[c]
                # one DMA per chunk covering all 4 k-tiles: HWDGE descriptor
                # generation is a serialized global resource (~625ns/DMA)
                nc.sync.dma_start(dsT_r[:, :, o:o + w], dsT_re[:, :, o:o + w])

            def load_ds8(s0, s1):
                nc.sync.dma_start(dhi_t[:, s0:s1], ds_hi.ap()[:, s0:s1])
                nc.sync.dma_start(dlo_t[:, s0:s1], ds_lo.ap()[:, s0:s1])

            # the DMA transfer device is globally serial: order transfers by
            # first use. dsT chunks feed phase 0 progressively; ds8 halves
            # must land before mm2(0) finishes (phase-1 end); xT tiles 2..7
            # are the least urgent (first used at phase 2).
            load_ds_chunk(0)
            nc.sync.dma_start(xT_r[:, :, 128:256], xT_re[:, :, 128:256])
            for c in range(1, nch):
                load_ds_chunk(c)
            nc.sync.dma_start(xT_r[:, :, 256:384], xT_re[:, :, 256:384])
            load_ds8(0, 13)
            load_ds8(13, NSLAB)
            nc.sync.dma_start(xT_r[:, :, 384:B], xT_re[:, :, 384:B])

            m_sb = res.tile([128, BT], F32, tag="m_sb")
            l_sb = res.tile([128, BT], F32, tag="l_sb")

            state = {}

            def emit_mm1_chunk(i, c, w):
                o = int(coff[c])
                gch, gmax = state[i][:2]
                g_ps = gps.tile([128, 512], F32, tag="gps")
                for k in range(KD):
                    nc.tensor.matmul(
                        g_ps[:, :w],
                        xT_r[:, k, i * 128:(i + 1) * 128],
                        dsT_r[:, k, o:o + w],
                        start=(k == 0), stop=False,
                    )
                nc.tensor.matmul(
                    g_ps[:, :w],
                    r1l_t[:, i * 128:(i + 1) * 128],
                    r1r_t[:, o:o + w],
                    start=False, stop=True,
                )
                G_c = gpool.tile([128, 512], F32, tag="G")
                nc.scalar.activation(G_c[:, :w], g_ps[:, :w],
                                     mybir.ActivationFunctionType.Copy)
                gch[c] = G_c
                maxq.add((i, c))

            maxq = set()

            def emit_max(i, c):
                if (i, c) not in maxq:
                    return
                maxq.discard((i, c))
                gch, gmax = state[i][:2]
                w = CHUNKS[c]
                nc.vector.reduce_max(gmax[:, c:c + 1], gch[c][:, :w], axis=X,
                                     op=mybir.AluOpType.max)

            def emit_exp_head(i):
                gch, gmax = state[i]
                gm = small.tile([128, 1], F32, tag="gm")
                nc.vector.reduce_max(gm[:], gmax[:], axis=X,
                                     op=mybir.AluOpType.max)
                nc.vector.tensor_mul(m_sb[:, i:i + 1], gm[:], c1_t[:, i:i + 1])
                nb = small.tile([128, 1], F32, tag="nb")
                nc.vector.tensor_scalar_mul(nb[:], m_sb[:, i:i + 1], -1.0)
                if i == BT - 1:
                    nc.gpsimd.dma_start(m_out.ap(), m_sb[:])
                lparts = small.tile([128, nch], F32, tag="lp")
                S_ps = spsum.tile([128, DSL, 128], F32, tag="S")
                state[i] = (gch, gmax, nb, lparts, S_ps)

            ptt = {}
            pcur = {}

            def emit_exp_chunk(i, c):
                gch, gmax, nb, lparts, S_ps = state[i]
                w = CHUNKS[c]
                pair = pair_of[c]
                off = off_in_pair[c]
                G_c = gch.pop(c)
                # chunks of a pair share one P tile so their u16-pair XBAR
                # transpose is a single DMA (HWDGE gen is globally serialized)
                if off == 0:
                    P2 = ppool.tile([128, 1024], FP8, tag="P")
                    pcur[(i, pair)] = P2
                else:
                    P2 = pcur[(i, pair)]
                nc.scalar.activation(
                    P2[:, off:off + w], G_c[:, :w],
                    mybir.ActivationFunctionType.Exp,
                    bias=nb[:], scale=c1_t[:, i:i + 1],
                    accum_out=lparts[:, c:c + 1],
                )
                if c == pair_last[pair]:
                    wp = off + w
                    pt = ptpool.tile([128, 4, 128], U16, tag="pt")
                    nc.sync.dma_start(pt[:, :wp // 256, :],
                                      P2.bitcast(U16)[:, :wp // 2],
                                      transpose=True)
                    ptt[(i, pair)] = pt
                    pcur.pop((i, pair))

            def emit_mm2_chunk(i, c, w):
                gch, gmax, nb, lparts, S_ps = state[i]
                pair = pair_of[c]
                pt = ptt[(i, pair)]
                s0 = int(coff[c]) // 256
                # the whole [128, 4, 128] tile is one PSUM bank = one zero
                # region: a single accumulation group (start on the global
                # first matmul, stop on the global last) — per-slice groups
                # would re-zero each other's partial sums.
                for j in range(w // 256):
                    s = s0 + j
                    blk = s - pair_slab0[pair]
                    ptj = pt[:, blk, :]
                    mov = ptj.bitcast(FP8).rearrange("p (b i) -> p i b", i=2)
                    for h, dst in enumerate((dhi_t, dlo_t)):
                        for e in range(DSL):
                            nc.tensor.matmul(
                                S_ps[:, e, :],
                                dst[:, s, :, e * 128:(e + 1) * 128],
                                mov,
                                start=(s == 0 and h == 0 and e == 0),
                                stop=(s == NSLAB - 1 and h == 1 and e == DSL - 1),
                                perf_mode=DR,
                                skip_group_check=True,
                            )
                if c == pair_last[pair]:
                    ptt.pop((i, pair))

            pending_stores = []

            def emit_mm2_tail(i):
                _, _, _, lparts, S_ps = state.pop(i)
                nc.vector.reduce_sum(l_sb[:, i:i + 1], lparts[:], axis=X,
                                     op=mybir.AluOpType.add)
                if i == BT - 1:
                    nc.gpsimd.dma_start(l_out.ap(), l_sb[:])
                S_sb = gpool.tile([128, 512], F32, tag="G")
                S_flat = S_ps[:].rearrange("p e b -> p (e b)")
                if i == BT - 1:
                    # split the final drain into pipelined halves
                    nc.vector.tensor_copy(S_sb[:, :256], S_flat[:, :256])
                    nc.sync.dma_start(
                        S_out.ap().rearrange("i p e b -> p i (e b)")[:, i, :256],
                        S_sb[:, :256])
                    nc.vector.tensor_copy(S_sb[:, 256:], S_flat[:, 256:])
                    nc.sync.dma_start(
                        S_out.ap().rearrange("i p e b -> p i (e b)")[:, i, 256:],
                        S_sb[:, 256:])
                else:
                    nc.vector.tensor_copy(S_sb[:], S_flat)
                    pending_stores.append((i, S_sb))

            def flush_stores():
                while pending_stores:
                    i, S_sb = pending_stores.pop(0)
                    nc.sync.dma_start(
                        S_out.ap().rearrange("i p e b -> p i (e b)")[:, i, :],
                        S_sb[:])

            def alloc_tile_state(i):
                gmax = small.tile([128, nch], F32, tag="gmax")
                state[i] = ({}, gmax)

            # software-pipelined, per phase i: mm1 of tile i leads, exp of
            # tile i-1 tracks it, mm2 of tile i-1 lags by LAG chunks so the
            # phase-boundary chain (exp_head -> exp -> DMA-transpose) is
            # covered by mm1 work on the PE
            LAG = 6
            import os
            LAG1 = int(os.environ.get("LAG1", "6"))   # phase-1 lag (tile 0)
            PRE = 3
            alloc_tile_state(0)
            if BT > 1:
                alloc_tile_state(1)
            for c, w in enumerate(CHUNKS):
                emit_mm1_chunk(0, c, w)
                if c < PRE:
                    emit_mm1_chunk(1, c, w)
                if c > 0:
                    emit_max(0, c - 1)
            emit_max(0, nch - 1)
            for i in range(1, BT):
                if i not in state:
                    alloc_tile_state(i)
                emit_exp_head(i - 1)
                sh = PRE   # head chunks of tile i ran in phase i-1
                for k in range(sh):
                    emit_max(i, k)
                lag = LAG1 if i == 1 else LAG
                for c in range(nch):
                    if c + sh < nch:
                        emit_mm1_chunk(i, c + sh, CHUNKS[c + sh])
                    elif i + 1 < BT and c + sh - nch < PRE:
                        # pull the next tile's first chunks into this phase
                        k = c + sh - nch
                        if i + 1 not in state:
                            alloc_tile_state(i + 1)
                        emit_mm1_chunk(i + 1, k, CHUNKS[k])
                    emit_exp_chunk(i - 1, c)
                    if c == 2:
                        flush_stores()
                    if c >= lag:
                        emit_mm2_chunk(i - 1, c - lag, CHUNKS[c - lag])
                    if c > 0 and c - 1 + sh < nch:
                        emit_max(i, c - 1 + sh)
                emit_max(i, nch - 1)
                if i == BT - 1:
                    emit_exp_head(BT - 1)
                    emit_exp_chunk(BT - 1, 0)
                    emit_exp_chunk(BT - 1, 1)
                for c in range(nch - lag, nch):
                    emit_mm2_chunk(i - 1, c, CHUNKS[c])
                emit_mm2_tail(i - 1)
            i = BT - 1
            for c, w in enumerate(CHUNKS):
                if c + 2 < nch:
                    emit_exp_chunk(i, c + 2)
                if c == 2:
                    flush_stores()
                emit_mm2_chunk(i, c, w)
            emit_mm2_tail(i)
            flush_stores()

    nc.compile()
    return nc


_NC_CACHE = {}


def _get_nc():
    if "nc" not in _NC_CACHE:
        _NC_CACHE["nc"] = _build()
    return _NC_CACHE["nc"]


def _split_bf16(v):
    hi = v.astype(np.float32).astype(BF)
    lo = (v.astype(np.float64) - hi.astype(np.float64)).astype(np.float32).astype(BF)
    return hi, lo


def _prep_inputs(x_t, t, dataset):
    x_t = np.asarray(x_t, dtype=np.float32)
    t = np.asarray(t, dtype=np.float32)
    dataset = np.asarray(dataset, dtype=np.float32)

    a = t.astype(np.float64)
    b = 1.0 - a
    c1 = np.ascontiguousarray(
        (a / (b * b)).astype(np.float32).reshape(BT, 128).T)

    dsp = np.full((NPAD, D), PADVAL, dtype=np.float32)
    dsp[:N] = dataset
    # beta must be consistent with the f16-rounded y used by mm1
    dsp16 = dsp.astype(np.float16).astype(np.float64)
    dsnc = ((dsp16 ** 2).sum(1) - float(D)).astype(np.float32)

    uu = -a / 2.0
    u_hi, u_lo = _split_bf16(uu)
    r1_lhsT = np.stack([u_hi, u_lo, u_hi]).astype(BF)       # (3, B)
    v_hi, v_lo = _split_bf16(dsnc.astype(np.float64))
    r1_rhs_full = np.stack([v_hi, v_hi, v_lo]).astype(BF)   # (3, NPAD)

    xT = np.ascontiguousarray(x_t.T).astype(np.float16).reshape(KD, 128, B)
    dsT_full = np.ascontiguousarray(dsp.T).astype(np.float16)  # (D, NPAD)

    # fp8 copies feed mm2 only; pad rows carry zero softmax weight but 2048
    # overflows e4m3 (max 448) into NaN, so zero them there.
    dsp8 = dsp.copy()
    dsp8[N:] = 0.0
    hi8 = dsp8.astype(E4)                                   # (NPAD, D)
    lo8 = (dsp8.astype(np.float64) - hi8.astype(np.float64)).astype(
        np.float32).astype(E4)

    in_maps = []
    for c in range(NCORES):
        sl = slice(c * NSH, (c + 1) * NSH)
        # DoubleRow pairing: n_local = 256*s + 2*p + i -> [p, s, i, d]
        hi_sh = np.ascontiguousarray(
            hi8[sl].reshape(NSLAB, 128, 2, D).transpose(1, 0, 2, 3))
        lo_sh = np.ascontiguousarray(
            lo8[sl].reshape(NSLAB, 128, 2, D).transpose(1, 0, 2, 3))
        im = {
            "xT": xT,
            "dsT": np.ascontiguousarray(dsT_full[:, sl]).reshape(KD, 128, NSH),
            "ds_hi": hi_sh,
            "ds_lo": lo_sh,
            "c1": c1,
            "r1_lhsT": r1_lhsT,
            "r1_rhs": np.ascontiguousarray(r1_rhs_full[:, sl]),
        }
        in_maps.append(im)
    return in_maps


def _combine_host(results, x_t, t):
    a = t.astype(np.float64)
    b = 1.0 - a
    m_c = np.stack([np.asarray(r["m_out"], dtype=np.float64).T.reshape(-1)
                    for r in results])                      # (8, B)
    l_c = np.stack([np.asarray(r["l_out"], dtype=np.float64).T.reshape(-1)
                    for r in results])                      # (8, B)
    # S_outT[i, p, e, bb] = S[b = i*128+bb, d = e*128+p]
    S_c = np.stack([np.asarray(r["S_outT"], dtype=np.float64)
                    .transpose(0, 3, 2, 1).reshape(B, D)
                    for r in results])                      # (8, B, D)
    M = m_c.max(0)
    w = np.exp(m_c - M)                                     # (8, B)
    S = np.einsum("cb,cbd->bd", w, S_c)
    L = (w * l_c).sum(0)
    wd = S / L[:, None]
    v = (-1.0 / b)[:, None] * x_t.astype(np.float64) \
        + (1.0 + a / b)[:, None] * wd
    return v.astype(np.float32)


def run_full(x_t, t, dataset, trace=False):
    nc = _get_nc()
    in_maps = _prep_inputs(x_t, t, dataset)
    res = run_bass_kernel_spmd(nc, in_maps, core_ids=list(range(NCORES)),
                               trace=trace)
    v = _combine_host(res.results, np.asarray(x_t, np.float32),
                      np.asarray(t, np.float32))
    return v, res


def kernel(x_t, t, dataset):
    v, _ = run_full(x_t, t, dataset)
    return v


# revision 26
# speedup vs baseline: 1.1149x; 1.0042x over previous
"""AnalyticGaussianVelocity Trainium2 kernel, 8 NeuronCores.

Math (reference):
    a=t, b=1-t
    logit_n = -(1/(2b^2)) * (|x|^2 - 2a x.y_n + a^2 |y_n|^2)
    v = -(1/b) x + (1 + a/b) * softmax(logit) @ dataset

Device-side per core (dataset sharded along N, padded 50000->51200, 6400/core,
free-dim chunks 12x512+256):
    G_n   = x.y_n + u*(|y_n|^2 - 512)   u=-a/2    (4 f32r matmuls -> PSUM +
                                                   a 5th 3-row split-bf16
                                                   rank-1 matmul)
    logit'_n = c1 * G_n, c1 = a/b^2
    m     = max_n logit'_n              (ACT Copy drain -> DVE chunk maxes)
    P_n   = exp(logit'_n - m)           (ACT exp, fp8-e4m3 out, f32 accum -> l)
    S^T   = sum_n P_n y_n               (fp8 DoubleRow matmuls: dataset is the
                                         stationary side split into e4m3
                                         hi+lo, P^T pairs are the moving side
                                         via u16 XBAR transposes of the fp8 P;
                                         out is S^T [d, b], 2x PE throughput
                                         vs bf16)
Host combine (flash-attention style) over the 8 core shards:
    M = max_c m_c; w = exp(m_c - M); S = sum w*S_c; L = sum w*l_c
    v = -(1/b) x + (1 + a/b) * S / L

Padding rows are the constant 2048.0 -> giant |y|^2 -> logit ~ -1e7 -> w 0.
The DoubleRow pairing groups n = 256*s + 2p + i (p = partition, i = k-group);
the u16 transpose of the fp8 P pairs adjacent n automatically, and the
dataset hi/lo tensors are pre-interleaved on the host to match.
"""

import numpy as np
import ml_dtypes

import concourse.bass as bass
from concourse import bacc
import concourse.mybir as mybir
import concourse.tile as tile
from concourse.bass_utils import run_bass_kernel_spmd

F32 = mybir.dt.float32
F32R = mybir.dt.float32r
F16 = mybir.dt.float16
BF16 = mybir.dt.bfloat16
FP8 = mybir.dt.float8e4
U16 = mybir.dt.uint16
BF = ml_dtypes.bfloat16
E4 = ml_dtypes.float8_e4m3
DR = mybir.MatmulPerfMode.DoubleRow

B, D, N = 1024, 512, 50000
NCORES = 8
NPAD = 51200                      # 8 * 6400, multiple of 2048
NSH = NPAD // NCORES              # 6400 per core
KD = D // 128                     # 4 contraction tiles for logits matmul
BT = B // 128                     # 8 batch tiles
CHUNKS = [256] + [512] * 12       # free-dim chunks of NSH (>=256: f32r rate;
                                  # small first chunk starts mm1 sooner)
# chunk pairs sharing one P tile / one XBAR transpose
PAIRS = [(0, 1), (2, 3), (4, 5), (6, 7), (8, 9), (10, 11), (12,)]
NSLAB = NSH // 256                # 25 DoubleRow slabs (256 n each)
DSL = D // 128                    # 4 d-slices for the S^T matmuls
PADVAL = 2048.0
X = mybir.AxisListType.X

GBUFS = 19                        # G chunks in flight (13/tile + slack)


def _build(combine=False):  # combine kept for test.py compat; host always combines
    nc = bacc.Bacc("TRN2", target_bir_lowering=False, debug=False,
                   num_devices=NCORES, dynamic_dma_scratch_size=512)

    xT = nc.declare_dram_parameter("xT", [KD, 128, B], F16, isOutput=False)
    dsT = nc.declare_dram_parameter("dsT", [KD, 128, NSH], F16, isOutput=False)
    ds_hi = nc.declare_dram_parameter("ds_hi", [128, NSLAB, 2, D], FP8,
                                      isOutput=False)
    ds_lo = nc.declare_dram_parameter("ds_lo", [128, NSLAB, 2, D], FP8,
                                      isOutput=False)
    c1d = nc.declare_dram_parameter("c1", [128, BT], F32, isOutput=False)
    r1l = nc.declare_dram_parameter("r1_lhsT", [3, B], BF16, isOutput=False)
    r1r = nc.declare_dram_parameter("r1_rhs", [3, NSH], BF16, isOutput=False)

    S_out = nc.declare_dram_parameter("S_outT", [BT, 128, DSL, 128], F32,
                                      isOutput=True)
    m_out = nc.declare_dram_parameter("m_out", [128, BT], F32, isOutput=True)
    l_out = nc.declare_dram_parameter("l_out", [128, BT], F32, isOutput=True)

    nch = len(CHUNKS)
    coff = np.concatenate([[0], np.cumsum(CHUNKS)])
    pair_of = {}
    off_in_pair = {}
    pair_last = {}
    pair_slab0 = {}
    for p, members in enumerate(PAIRS):
        o = 0
        for c in members:
            pair_of[c] = p
            off_in_pair[c] = o
            o += CHUNKS[c]
        pair_last[p] = members[-1]
        pair_slab0[p] = int(coff[members[0]]) // 256
        assert o <= 1024

    with tile.TileContext(nc) as tc:
        with (
            tc.tile_pool(name="res", bufs=1) as res,
            tc.tile_pool(name="gpool", bufs=GBUFS) as gpool,
            tc.tile_pool(name="ppool", bufs=8) as ppool,
            tc.tile_pool(name="small", bufs=2) as small,
            tc.tile_pool(name="ptpool", bufs=8) as ptpool,
            tc.tile_pool(name="gps", bufs=4, space="PSUM") as gps,
            tc.tile_pool(name="spsum", bufs=2, space="PSUM") as spsum,
        ):
            # ---- residents (DMA in first-use order; smalls off sync queue) --
            c1_t = res.tile([128, BT], F32, tag="c1")
            nc.gpsimd.dma_start(c1_t[:], c1d[:])
            r1l_t = res.tile([3, B], BF16, tag="r1l")
            nc.gpsimd.dma_start(r1l_t[:], r1l[:])
            r1r_t = res.tile([3, NSH], BF16, tag="r1r")
            nc.gpsimd.dma_start(r1r_t[:], r1r[:])

            xT_r = res.tile([128, KD, B], F16, tag="xT_r")
            xT_re = xT.ap().rearrange("k p b -> p k b")
            # tile 0 leads; tile 1 (PRE) follows right after chunk 0
            nc.sync.dma_start(xT_r[:, :, 0:128], xT_re[:, :, 0:128])

            dsT_r = res.tile([128, KD, NSH], F16, tag="dsT_r")
            dsT_re = dsT.ap().rearrange("k p n -> p k n")
            dhi_t = res.tile([128, NSLAB, 2, D], FP8, tag="dhi")
            dlo_t = res.tile([128, NSLAB, 2, D], FP8, tag="dlo")

            def load_ds_chunk(c):
                o = int(coff[c])
                w = CHUNKS[c]
                # one DMA per chunk covering all 4 k-tiles: HWDGE descriptor
                # generation is a serialized global resource (~625ns/DMA)
                nc.sync.dma_start(dsT_r[:, :, o:o + w], dsT_re[:, :, o:o + w])

            def load_ds8(s0, s1):
                nc.sync.dma_start(dhi_t[:, s0:s1], ds_hi.ap()[:, s0:s1])
                nc.sync.dma_start(dlo_t[:, s0:s1], ds_lo.ap()[:, s0:s1])

            # the DMA transfer device is globally serial: order transfers by
            # first use. dsT chunks feed phase 0 progressively; ds8 halves
            # must land before mm2(0) finishes (phase-1 end); xT tiles 2..7
            # are the least urgent (first used at phase 2).
            load_ds_chunk(0)
            nc.sync.dma_start(xT_r[:, :, 128:256], xT_re[:, :, 128:256])
            for c in range(1, nch):
                load_ds_chunk(c)
            nc.sync.dma_start(xT_r[:, :, 256:384], xT_re[:, :, 256:384])
            load_ds8(0, 13)
            load_ds8(13, NSLAB)
            nc.sync.dma_start(xT_r[:, :, 384:B], xT_re[:, :, 384:B])

            m_sb = res.tile([128, BT], F32, tag="m_sb")
            l_sb = res.tile([128, BT], F32, tag="l_sb")

            state = {}

            def emit_mm1_chunk(i, c, w):
                o = int(coff[c])
                gch, gmax = state[i][:2]
                g_ps = gps.tile([128, 512], F32, tag="gps")
                for k in range(KD):
                    nc.tensor.matmul(
                        g_ps[:, :w],
                        xT_r[:, k, i * 128:(i + 1) * 128],
                        dsT_r[:, k, o:o + w],
                        start=(k == 0), stop=False,
                    )
                nc.tensor.matmul(
                    g_ps[:, :w],
                    r1l_t[:, i * 128:(i + 1) * 128],
                    r1r_t[:, o:o + w],
                    start=False, stop=True,
                )
                G_c = gpool.tile([128, 512], F32, tag="G")
                if i == BT - 1:
                    # phase 7 is ACT-single-file (drains + two exp chains);
                    # drain the last tile on the otherwise-idle DVE
                    nc.vector.tensor_copy(G_c[:, :w], g_ps[:, :w])
                else:
                    nc.scalar.activation(G_c[:, :w], g_ps[:, :w],
                                         mybir.ActivationFunctionType.Copy)
                gch[c] = G_c
                maxq.add((i, c))

            maxq = set()

            def emit_max(i, c):
                if (i, c) not in maxq:
                    return
                maxq.discard((i, c))
                gch, gmax = state[i][:2]
                w = CHUNKS[c]
                nc.vector.reduce_max(gmax[:, c:c + 1], gch[c][:, :w], axis=X,
                                     op=mybir.AluOpType.max)

            def emit_exp_head(i):
                gch, gmax = state[i]
                gm = small.tile([128, 1], F32, tag="gm")
                nc.vector.reduce_max(gm[:], gmax[:], axis=X,
                                     op=mybir.AluOpType.max)
                nc.vector.tensor_mul(m_sb[:, i:i + 1], gm[:], c1_t[:, i:i + 1])
                nb = small.tile([128, 1], F32, tag="nb")
                nc.vector.tensor_scalar_mul(nb[:], m_sb[:, i:i + 1], -1.0)
                if i == BT - 1:
                    nc.gpsimd.dma_start(m_out.ap(), m_sb[:])
                lparts = small.tile([128, nch], F32, tag="lp")
                S_ps = spsum.tile([128, DSL, 128], F32, tag="S")
                state[i] = (gch, gmax, nb, lparts, S_ps)

            ptt = {}
            pcur = {}

            def emit_exp_chunk(i, c):
                gch, gmax, nb, lparts, S_ps = state[i]
                w = CHUNKS[c]
                pair = pair_of[c]
                off = off_in_pair[c]
                G_c = gch.pop(c)
                # chunks of a pair share one P tile so their u16-pair XBAR
                # transpose is a single DMA (HWDGE gen is globally serialized)
                if off == 0:
                    P2 = ppool.tile([128, 1024], FP8, tag="P")
                    pcur[(i, pair)] = P2
                else:
                    P2 = pcur[(i, pair)]
                nc.scalar.activation(
                    P2[:, off:off + w], G_c[:, :w],
                    mybir.ActivationFunctionType.Exp,
                    bias=nb[:], scale=c1_t[:, i:i + 1],
                    accum_out=lparts[:, c:c + 1],
                )
                if c == pair_last[pair]:
                    wp = off + w
                    pt = ptpool.tile([128, 4, 128], U16, tag="pt")
                    nc.sync.dma_start(pt[:, :wp // 256, :],
                                      P2.bitcast(U16)[:, :wp // 2],
                                      transpose=True)
                    ptt[(i, pair)] = pt
                    pcur.pop((i, pair))

            def emit_mm2_chunk(i, c, w):
                gch, gmax, nb, lparts, S_ps = state[i]
                pair = pair_of[c]
                pt = ptt[(i, pair)]
                s0 = int(coff[c]) // 256
                # the whole [128, 4, 128] tile is one PSUM bank = one zero
                # region: a single accumulation group (start on the global
                # first matmul, stop on the global last) — per-slice groups
                # would re-zero each other's partial sums.
                for j in range(w // 256):
                    s = s0 + j
                    blk = s - pair_slab0[pair]
                    ptj = pt[:, blk, :]
                    mov = ptj.bitcast(FP8).rearrange("p (b i) -> p i b", i=2)
                    for h, dst in enumerate((dhi_t, dlo_t)):
                        for e in range(DSL):
                            nc.tensor.matmul(
                                S_ps[:, e, :],
                                dst[:, s, :, e * 128:(e + 1) * 128],
                                mov,
                                start=(s == 0 and h == 0 and e == 0),
                                stop=(s == NSLAB - 1 and h == 1 and e == DSL - 1),
                                perf_mode=DR,
                                skip_group_check=True,
                            )
                if c == pair_last[pair]:
                    ptt.pop((i, pair))

            pending_stores = []

            def emit_mm2_tail(i):
                _, _, _, lparts, S_ps = state.pop(i)
                nc.vector.reduce_sum(l_sb[:, i:i + 1], lparts[:], axis=X,
                                     op=mybir.AluOpType.add)
                if i == BT - 1:
                    nc.gpsimd.dma_start(l_out.ap(), l_sb[:])
                S_sb = gpool.tile([128, 512], F32, tag="G")
                S_flat = S_ps[:].rearrange("p e b -> p (e b)")
                if i == BT - 1:
                    # split the final drain into pipelined halves
                    nc.vector.tensor_copy(S_sb[:, :256], S_flat[:, :256])
                    nc.sync.dma_start(
                        S_out.ap().rearrange("i p e b -> p i (e b)")[:, i, :256],
                        S_sb[:, :256])
                    nc.vector.tensor_copy(S_sb[:, 256:], S_flat[:, 256:])
                    nc.sync.dma_start(
                        S_out.ap().rearrange("i p e b -> p i (e b)")[:, i, 256:],
                        S_sb[:, 256:])
                else:
                    nc.vector.tensor_copy(S_sb[:], S_flat)
                    pending_stores.append((i, S_sb))

            def flush_stores():
                while pending_stores:
                    i, S_sb = pending_stores.pop(0)
                    nc.sync.dma_start(
                        S_out.ap().rearrange("i p e b -> p i (e b)")[:, i, :],
                        S_sb[:])

            def alloc_tile_state(i):
                gmax = small.tile([128, nch], F32, tag="gmax")
                state[i] = ({}, gmax)

            # software-pipelined, per phase i: mm1 of tile i leads, exp of
            # tile i-1 tracks it, mm2 of tile i-1 lags by LAG chunks so the
            # phase-boundary chain (exp_head -> exp -> DMA-transpose) is
            # covered by mm1 work on the PE
            LAG = 6
            import os
            LAG1 = int(os.environ.get("LAG1", "6"))   # phase-1 lag (tile 0)
            PRE = 3
            alloc_tile_state(0)
            if BT > 1:
                alloc_tile_state(1)
            for c, w in enumerate(CHUNKS):
                emit_mm1_chunk(0, c, w)
                if c < PRE:
                    emit_mm1_chunk(1, c, w)
                if c > 0:
                    emit_max(0, c - 1)
            emit_max(0, nch - 1)
            for i in range(1, BT):
                if i not in state:
                    alloc_tile_state(i)
                emit_exp_head(i - 1)
                sh = PRE   # head chunks of tile i ran in phase i-1
                for k in range(sh):
                    emit_max(i, k)
                lag = LAG1 if i == 1 else LAG
                for c in range(nch):
                    if c + sh < nch:
                        emit_mm1_chunk(i, c + sh, CHUNKS[c + sh])
                    elif i + 1 < BT and c + sh - nch < PRE:
                        # pull the next tile's first chunks into this phase
                        k = c + sh - nch
                        if i + 1 not in state:
                            alloc_tile_state(i + 1)
                        emit_mm1_chunk(i + 1, k, CHUNKS[k])
                    emit_exp_chunk(i - 1, c)
                    if c == 2:
                        flush_stores()
                    if c >= lag:
                        emit_mm2_chunk(i - 1, c - lag, CHUNKS[c - lag])
                    if c > 0 and c - 1 + sh < nch:
                        emit_max(i, c - 1 + sh)
                emit_max(i, nch - 1)
                if i == BT - 1:
                    emit_exp_head(BT - 1)
                    emit_exp_chunk(BT - 1, 0)
                    emit_exp_chunk(BT - 1, 1)
                for c in range(nch - lag, nch):
                    emit_mm2_chunk(i - 1, c, CHUNKS[c])
                emit_mm2_tail(i - 1)
            i = BT - 1
            for c, w in enumerate(CHUNKS):
                if c + 2 < nch:
                    emit_exp_chunk(i, c + 2)
                if c == 2:
                    flush_stores()
                emit_mm2_chunk(i, c, w)
            emit_mm2_tail(i)
            flush_stores()

    nc.compile()
    return nc


_NC_CACHE = {}


def _get_nc():
    if "nc" not in _NC_CACHE:
        _NC_CACHE["nc"] = _build()
    return _NC_CACHE["nc"]


def _split_bf16(v):
    hi = v.astype(np.float32).astype(BF)
    lo = (v.astype(np.float64) - hi.astype(np.float64)).astype(np.float32).astype(BF)
    return hi, lo


def _prep_inputs(x_t, t, dataset):
    x_t = np.asarray(x_t, dtype=np.float32)
    t = np.asarray(t, dtype=np.float32)
    dataset = np.asarray(dataset, dtype=np.float32)

    a = t.astype(np.float64)
    b = 1.0 - a
    c1 = np.ascontiguousarray(
        (a / (b * b)).astype(np.float32).reshape(BT, 128).T)

    dsp = np.full((NPAD, D), PADVAL, dtype=np.float32)
    dsp[:N] = dataset
    # beta must be consistent with the f16-rounded y used by mm1
    dsp16 = dsp.astype(np.float16).astype(np.float64)
    dsnc = ((dsp16 ** 2).sum(1) - float(D)).astype(np.float32)

    uu = -a / 2.0
    u_hi, u_lo = _split_bf16(uu)
    r1_lhsT = np.stack([u_hi, u_lo, u_hi]).astype(BF)       # (3, B)
    v_hi, v_lo = _split_bf16(dsnc.astype(np.float64))
    r1_rhs_full = np.stack([v_hi, v_hi, v_lo]).astype(BF)   # (3, NPAD)

    xT = np.ascontiguousarray(x_t.T).astype(np.float16).reshape(KD, 128, B)
    dsT_full = np.ascontiguousarray(dsp.T).astype(np.float16)  # (D, NPAD)

    # fp8 copies feed mm2 only; pad rows carry zero softmax weight but 2048
    # overflows e4m3 (max 448) into NaN, so zero them there.
    dsp8 = dsp.copy()
    dsp8[N:] = 0.0
    hi8 = dsp8.astype(E4)                                   # (NPAD, D)
    lo8 = (dsp8.astype(np.float64) - hi8.astype(np.float64)).astype(
        np.float32).astype(E4)

    in_maps = []
    for c in range(NCORES):
        sl = slice(c * NSH, (c + 1) * NSH)
        # DoubleRow pairing: n_local = 256*s + 2*p + i -> [p, s, i, d]
        hi_sh = np.ascontiguousarray(
            hi8[sl].reshape(NSLAB, 128, 2, D).transpose(1, 0, 2, 3))
        lo_sh = np.ascontiguousarray(
            lo8[sl].reshape(NSLAB, 128, 2, D).transpose(1, 0, 2, 3))
        im = {
            "xT": xT,
            "dsT": np.ascontiguousarray(dsT_full[:, sl]).reshape(KD, 128, NSH),
            "ds_hi": hi_sh,
            "ds_lo": lo_sh,
            "c1": c1,
            "r1_lhsT": r1_lhsT,
            "r1_rhs": np.ascontiguousarray(r1_rhs_full[:, sl]),
        }
        in_maps.append(im)
    return in_maps


def _combine_host(results, x_t, t):
    a = t.astype(np.float64)
    b = 1.0 - a
    m_c = np.stack([np.asarray(r["m_out"], dtype=np.float64).T.reshape(-1)
                    for r in results])                      # (8, B)
    l_c = np.stack([np.asarray(r["l_out"], dtype=np.float64).T.reshape(-1)
                    for r in results])                      # (8, B)
    # S_outT[i, p, e, bb] = S[b = i*128+bb, d = e*128+p]
    S_c = np.stack([np.asarray(r["S_outT"], dtype=np.float64)
                    .transpose(0, 3, 2, 1).reshape(B, D)
                    for r in results])                      # (8, B, D)
    M = m_c.max(0)
    w = np.exp(m_c - M)                                     # (8, B)
    S = np.einsum("cb,cbd->bd", w, S_c)
    L = (w * l_c).sum(0)
    wd = S / L[:, None]
    v = (-1.0 / b)[:, None] * x_t.astype(np.float64) \
        + (1.0 + a / b)[:, None] * wd
    return v.astype(np.float32)


def run_full(x_t, t, dataset, trace=False):
    nc = _get_nc()
    in_maps = _prep_inputs(x_t, t, dataset)
    res = run_bass_kernel_spmd(nc, in_maps, core_ids=list(range(NCORES)),
                               trace=trace)
    v = _combine_host(res.results, np.asarray(x_t, np.float32),
                      np.asarray(t, np.float32))
    return v, res


def kernel(x_t, t, dataset):
    v, _ = run_full(x_t, t, dataset)
    return v
